# revision 17
# baseline (speedup 1.0000x reference)
"""Trainium2 Bass kernel for nn_MultiHeadCDGCN — bf16 dataflow version.

Math (per batch b, one core per batch):
  t_w  = softmax(x, axis=T);  TAtt = sum_T(x * t_w)          [N, D]
  Q    = x @ W_Q.T                                           [T, N, D]
  K    = TAtt @ W_K.T ; V = TAtt @ W_V.T                     [N, D]
  S_th = Q_th @ K_h.T / sqrt(dh)   (per t, head h)           [N, N]
  out  = (relu(S) + I) @ V = relu(S) @ V + V                 [T, N, D]

Differences vs the fp32 kernel:
  - All attention operands (x.T, Q.T, K.T, V, A) are bf16: matmuls run
    at 1 cycle/row and PSUM-evacuation bandwidth halves on SBUF writes.
  - Stats (sum_e / sum_xe) accumulate in fp16 on DVE (2x mode).
  - Output path: po -> (+V, ->bf16) -> PE transpose -> PSUM bf16 ->
    SBUF -> single casting DMA (bf16->fp32) per chunk with 1KB
    contiguous HBM lines.
"""

import sys

import numpy as np

sys.path.insert(0, "/opt/trn_rl_repo")

import concourse.bacc as bacc  # noqa: E402
import concourse.tile as tile  # noqa: E402
from concourse import mybir  # noqa: E402
from concourse.masks import make_identity  # noqa: E402
from concourse.bass_utils import run_bass_kernel_spmd  # noqa: E402

F32 = mybir.dt.float32
F32R = mybir.dt.float32r
BF16 = mybir.dt.bfloat16
FP16 = mybir.dt.float16
AF = mybir.ActivationFunctionType
ALU = mybir.AluOpType

B, T, N, D, H, DH = 8, 32, 256, 256, 8, 32
P = 128
NCHUNKS = 16
CHUNK_T = 2
CHUNK_TN = CHUNK_T * N  # 512

_CACHE: dict = {}


def _build_program():
    nc = bacc.Bacc()

    x_d = nc.dram_tensor("x", [T, N, D], F32, kind="ExternalInput")
    wqt_d = nc.dram_tensor("wqt", [D, D], F32, kind="ExternalInput")
    wkt_d = nc.dram_tensor("wkt", [D, D], F32, kind="ExternalInput")
    wvt_d = nc.dram_tensor("wvt", [D, D], F32, kind="ExternalInput")
    out_d = nc.dram_tensor("out", [T, N, D], F32, kind="ExternalOutput")

    with tile.TileContext(nc) as tc:
        with (
            tc.tile_pool(name="consts", bufs=1) as consts,
            tc.tile_pool(name="xa", bufs=3) as xa_pool,
            tc.tile_pool(name="ew", bufs=6) as e_pool,
            tc.tile_pool(name="at", bufs=18) as a_pool,
            tc.tile_pool(name="ot", bufs=4) as o_pool,
            tc.tile_pool(name="oo", bufs=3) as oo_pool,
            tc.tile_pool(name="misc", bufs=2) as misc,
            tc.tile_pool(name="ps_a", bufs=3, space="PSUM") as ps_a,
            tc.tile_pool(name="ps_o", bufs=2, space="PSUM") as ps_o,
        ):
            eye = consts.tile([P, P], BF16)
            make_identity(nc, eye)
            eye_f = consts.tile([P, P], F32)
            make_identity(nc, eye_f)

            # Weights [k, j] cast fp32->bf16 in-flight on the gpsimd
            # software-DGE DMA; keeps the sync queue free for x prefetch.
            wqt_sb = consts.tile([P, 2, D], BF16)
            wkt_sb = consts.tile([P, 2, D], BF16)
            wvt_sb = consts.tile([P, 2, D], BF16)
            for w_sb, w_d in ((wqt_sb, wqt_d), (wkt_sb, wkt_d), (wvt_sb, wvt_d)):
                for kc in range(2):
                    nc.gpsimd.dma_start(
                        out=w_sb[:, kc, :],
                        in_=w_d[kc * P : (kc + 1) * P, :],
                    )

            # Softmax-pool statistics in transposed [d, n] layout, fp16.
            sum_e = consts.tile([P, 2, N], FP16)
            sum_xe = consts.tile([P, 2, N], FP16)
            nc.gpsimd.memset(sum_e, 0.0)
            nc.gpsimd.memset(sum_xe, 0.0)

            # Q.T strip [j, tn] resident, bf16 (4 MB).
            qt_sb = consts.tile([P, 2, T * N], BF16)
            # x.T is not kept; only per-chunk.

            # ---------------- Phase A: stream x, x.T, stats, Q.T
            for c in range(NCHUNKS):
                t0 = c * CHUNK_T
                xa = xa_pool.tile([P, 4, D], F32)
                xav = x_d[t0 : t0 + CHUNK_T].rearrange(
                    "t (s p) d -> p (t s) d", p=P
                )
                nc.sync.dma_start(out=xa[:, 0:2, :], in_=xav[:, 0:2, :])
                nc.scalar.dma_start(out=xa[:, 2:4, :], in_=xav[:, 2:4, :])

                # Transpose x chunk -> pt PSUM fp32 [d-half, tn] per dc.
                xt = e_pool.tile([P, 2, CHUNK_TN], BF16, name="xt")
                e_t = e_pool.tile([P, 2, CHUNK_TN], BF16, name="e_t")
                xe_t = e_pool.tile([P, 2, CHUNK_TN], BF16, name="xe_t")
                for dc in range(2):
                    pt = ps_a.tile([P, CHUNK_TN], F32, tag="psa", name=f"pt{dc}")
                    for s in range(4):
                        nc.tensor.transpose(
                            pt[:, s * P : (s + 1) * P],
                            xa[:, s, dc * P : (dc + 1) * P],
                            eye_f,
                        )
                    # Evac to bf16 x.T (ACT) + exp (ACT).
                    nc.scalar.activation(xt[:, dc, :], pt, AF.Copy)
                    nc.scalar.activation(e_t[:, dc, :], pt, AF.Exp)
                # xe = x * e (DVE, all-SBUF bf16).
                nc.vector.tensor_mul(xe_t, xt, e_t)
                # Stats accumulate over the 2 frames (DVE fp16 2x mode).
                ev = e_t.rearrange("p dc (t n) -> p dc t n", t=CHUNK_T)
                xev = xe_t.rearrange("p dc (t n) -> p dc t n", t=CHUNK_T)
                for ti in range(CHUNK_T):
                    nc.vector.tensor_add(sum_e, sum_e, ev[:, :, ti, :])
                    nc.vector.tensor_add(sum_xe, sum_xe, xev[:, :, ti, :])

                # Q.T chunk [j, tn]: both jc halves into one 2-bank tile.
                pq = ps_a.tile([P, 2 * CHUNK_TN], F32, tag="psa", name="pq")
                for jc in range(2):
                    for kc in range(2):
                        nc.tensor.matmul(
                            pq[:, jc * CHUNK_TN : (jc + 1) * CHUNK_TN],
                            wqt_sb[:, kc, jc * P : (jc + 1) * P],
                            xt[:, kc, :],
                            start=(kc == 0),
                            stop=(kc == 1),
                        )
                nc.vector.tensor_copy(
                    qt_sb[:, :, c * CHUNK_TN : (c + 1) * CHUNK_TN],
                    pq.rearrange("p (jc tn) -> p jc tn", jc=2),
                )

            # ---------------- Phase B: TAtt.T, K.T, V, V.T
            rec = misc.tile([P, 2, N], F32)
            tatt_t = consts.tile([P, 2, N], BF16)  # TAtt.T [d, n]
            for dc in range(2):
                nc.vector.reciprocal(rec[:, dc, :], sum_e[:, dc, :])
                nc.vector.tensor_mul(
                    tatt_t[:, dc, :], sum_xe[:, dc, :], rec[:, dc, :]
                )

            kt_sb = consts.tile([P, 2, N], BF16)  # K.T [j, m] (pre-scaled)
            for jc in range(2):
                pk = ps_a.tile([P, N], F32, tag="psa", name="pk")
                for kc in range(2):
                    nc.tensor.matmul(
                        pk,
                        wkt_sb[:, kc, jc * P : (jc + 1) * P],
                        tatt_t[:, kc, :],
                        start=(kc == 0),
                        stop=(kc == 1),
                    )
                nc.vector.tensor_copy(kt_sb[:, jc, :], pk)

            v_sb = consts.tile([P, 2, D], BF16)  # V [m, j]
            for mc in range(2):
                pv = ps_a.tile([P, D], F32, tag="psa", name="pv")
                for kc in range(2):
                    nc.tensor.matmul(
                        pv,
                        tatt_t[:, kc, mc * P : (mc + 1) * P],
                        wvt_sb[:, kc, :],
                        start=(kc == 0),
                        stop=(kc == 1),
                    )
                nc.vector.tensor_copy(v_sb[:, mc, :], pv)

            # V.T doubled over t for 512-wide +V evac: [j, hg, t, m-block?]
            # vt_dbl[p, hg, ti, m] = V.T[hg*128+p, m]
            vt_dbl = consts.tile([P, 2, CHUNK_T, N], BF16)
            for jc in range(2):
                pt2f = ps_a.tile([P, N], BF16, tag="psa", name="pt2")
                for mc in range(2):
                    nc.tensor.transpose(
                        pt2f[:, mc * P : (mc + 1) * P],
                        v_sb[:, mc, jc * P : (jc + 1) * P],
                        eye,
                    )
                for ti in range(CHUNK_T):
                    nc.scalar.activation(vt_dbl[:, jc, ti, :], pt2f, AF.Copy)

            # ---------------- Phase C: attention + output
            # Chunk-PAIR bursts: S for both chunks (row-tiling mode), then
            # A@V for both (col-tiling), then output transposes for both
            # (full-array) - halves PE tiling-mode switches, each of which
            # drains the PE array.
            for cp in range(NCHUNKS // 2):
                a_str = {}
                nrelu = 0
                for c in (2 * cp, 2 * cp + 1):
                    for hg in range(2):
                        for mc in range(2):
                            for rp in range(2):
                                ps2 = ps_a.tile(
                                    [P, 2 * CHUNK_TN],
                                    F32,
                                    tag="psa",
                                    name=f"ps{hg}{mc}{rp}",
                                )
                                for rh in range(2):
                                    r = rp * 2 + rh
                                    nc.tensor.matmul(
                                        ps2[
                                            :,
                                            rh * CHUNK_TN : (rh + 1) * CHUNK_TN,
                                        ],
                                        kt_sb[
                                            r * 32 : (r + 1) * 32,
                                            hg,
                                            mc * P : (mc + 1) * P,
                                        ],
                                        qt_sb[
                                            r * 32 : (r + 1) * 32,
                                            hg,
                                            c * CHUNK_TN : (c + 1) * CHUNK_TN,
                                        ],
                                        start=True,
                                        stop=True,
                                        tile_position=(r * 32, 0),
                                    )
                                a2 = a_pool.tile(
                                    [P, 2 * CHUNK_TN],
                                    BF16,
                                    tag="at",
                                    name=f"a{c % 2}{hg}{mc}{rp}",
                                )
                                # Split relu evacuation DVE:ACT at 3:5.
                                if (c + nrelu) % 8 in (0, 3, 6):
                                    nc.vector.tensor_scalar_max(a2, ps2, 0.0)
                                else:
                                    nc.scalar.activation(a2, ps2, AF.Relu)
                                nrelu += 1
                                for rh in range(2):
                                    a_str[(c, hg, rp * 2 + rh, mc)] = a2[
                                        :, rh * CHUNK_TN : (rh + 1) * CHUNK_TN
                                    ]
                o_ts = {}
                for c in (2 * cp, 2 * cp + 1):
                    o_t = o_pool.tile(
                        [P, 2, CHUNK_TN], BF16, name=f"o_t{c % 2}"
                    )
                    o_ts[c] = o_t
                    for hg in range(2):
                        po = ps_o.tile(
                            [P, CHUNK_TN], F32, tag="po", name=f"po{hg}"
                        )
                        for mc in range(2):
                            for r in range(4):
                                h = hg * 4 + r
                                nc.tensor.matmul(
                                    po[r * 32 : (r + 1) * 32, :],
                                    v_sb[:, mc, h * 32 : (h + 1) * 32],
                                    a_str[(c, hg, r, mc)],
                                    start=(mc == 0),
                                    stop=(mc == 1),
                                    tile_position=(0, r * 32),
                                    skip_group_check=True,
                                )
                        # Evac + add V (self-loop), 512-wide DVE -> bf16.
                        nc.vector.scalar_tensor_tensor(
                            out=o_t[:, hg, :],
                            in0=po,
                            scalar=1.0,
                            in1=vt_dbl.rearrange("p hg t n -> p hg (t n)")[
                                :, hg, :
                            ],
                            op0=ALU.mult,
                            op1=ALU.add,
                        )
                for c in (2 * cp, 2 * cp + 1):
                    t0 = c * CHUNK_T
                    otv = o_ts[c].rearrange("p hg (t n) -> p hg t n", t=CHUNK_T)
                    o_out = oo_pool.tile(
                        [P, CHUNK_T, 2, D], BF16, name=f"o_out{c % 2}"
                    )
                    for nc2 in range(2):
                        pso = ps_o.tile(
                            [P, CHUNK_T, D], BF16, tag="po", name=f"pso{nc2}"
                        )
                        for ti in range(CHUNK_T):
                            for hg in range(2):
                                nc.tensor.transpose(
                                    pso[:, ti, hg * P : (hg + 1) * P],
                                    otv[:, hg, ti, nc2 * P : (nc2 + 1) * P],
                                    eye,
                                )
                        # Evac bf16 PSUM -> SBUF (DVE 2x).
                        nc.vector.tensor_copy(o_out[:, :, nc2, :], pso)
                    # One casting DMA per chunk: bf16 -> fp32, 1KB lines.
                    nc.gpsimd.dma_start(
                        out=out_d[t0 : t0 + CHUNK_T].rearrange(
                            "t (nc2 p) d -> p (t nc2) d", p=P
                        ),
                        in_=o_out.rearrange("p t nc2 d -> p (t nc2) d"),
                    )

    nc.finalize()
    return nc


def kernel(**inputs) -> np.ndarray:
    x = np.ascontiguousarray(np.asarray(inputs["x"], dtype=np.float32))
    w_q = np.asarray(inputs["W_Q"], dtype=np.float32)
    w_k = np.asarray(inputs["W_K"], dtype=np.float32)
    w_v = np.asarray(inputs["W_V"], dtype=np.float32)

    if "nc" not in _CACHE:
        _CACHE["nc"] = _build_program()
    nc = _CACHE["nc"]

    wqt = np.ascontiguousarray(w_q.T)
    wkt = np.ascontiguousarray(w_k.T) * np.float32(1.0 / np.sqrt(DH))
    wvt = np.ascontiguousarray(w_v.T)

    in_maps = [
        {"x": np.ascontiguousarray(x[b]), "wqt": wqt, "wkt": wkt, "wvt": wvt}
        for b in range(B)
    ]
    res = run_bass_kernel_spmd(nc, in_maps, core_ids=list(range(B)))
    out = np.stack([res.results[b]["out"] for b in range(B)], axis=0)
    return out.reshape(B, T, N, D)
